# revision 21
# baseline (speedup 1.0000x reference)
"""BiLSTM single-step kernel for 8 Trainium2 NeuronCores.

Math per direction d (f, b):
    gates    = x_d @ Wx_d^T + h_d @ Wh_d^T + b_d          # [4096, 4*1024]
    f,i,o    = sigmoid(...), C = tanh(...)
    c_new    = f*c + i*C ; h_new = o*tanh(c_new)

Distribution: data-parallel over batch, 512 rows per core; weights
replicated. Per core each direction is a [512, 2048] x [2048, 4096] GEMM.

Precision strategy: the x-part (|x|~1) runs in fp16 for the C/i/o gates;
the h-part is tiny (|h|~0.02, |h.Wh| ~ 2% of the gate magnitude) and runs
in fp8-e5m2 with DoubleRow perf mode (2 k-chunks per matmul instruction),
accumulating into the same fp32 PSUM bank. The f gate's x-part ALSO runs
in fp8-e5m2 DoubleRow: its gate output only multiplies the tiny cell
state (|c| <= ~0.1), so its error sensitivity is ~10x lower than the
other gates (CPU-verified end-to-end relmax ~5e-3, vs the 2e-2 budget).

On-chip layout is the transpose of the reference: psum tiles are
gates^T [128 gate-hidden partitions, 512 batch], so the per-(gate,h) bias
is per-partition (fused into the scalar-engine sigmoid/tanh) and the
contraction index i sits on SBUF partitions for both matmul operands.
All transposes happen host-side in numpy.

Schedule details (from perfetto analysis of earlier revisions):
  - The DMA rings take ~9 us to start flowing and are per-transfer
    latency-bound while ramping, so startup transfers are few and big
    (except a small first slice per ring to cut first-matmul latency).
  - Group 0 is inherently DMA-bound (~3.4 MB of weights+activations vs
    ~9.5 us of matmul time): it runs all four gates' fp8 h-parts first
    (cheapest data), then the f gate's fp8 x-part, then the fp16
    x-parts, with dummy matmuls on a zeroed tile interleaved so the
    PE_HAM clock gate never sees an idle window (a re-throttle to
    1.2 GHz costs ~3-5 us; the dummies run where the PE would stall).
  - ~10 dummy matmuls also run during the initial pure-DMA fill so the
    PE is already at 2.4 GHz when the first real matmul issues.
  - Per-group fp8 weights ride in one 512 KB transfer (4 KB contiguous
    lines) instead of four 128 KB ones (1 KB lines).
  - The final group's c/h stores are split across the three DMA rings
    instead of serializing ~512 KB on one ring at the kernel tail.
"""

import numpy as np
import ml_dtypes

import concourse.bass as bass
import concourse.mybir as mybir
import concourse.tile as tile
from concourse import bacc, bass_utils
from concourse.bass import ts

BATCH, IN, HID = 4096, 1024, 1024
NCORES = 8
BS = BATCH // NCORES          # 512 batch rows per core = matmul free dim N
KX = IN // 128                # 8 contraction chunks (x part)
KH = HID // 128               # 8 fp8 contraction chunks (h part)
HC = HID // 128               # 8 hidden chunks of 128

F16 = mybir.dt.float16
F8 = mybir.dt.float8e5
F32 = mybir.dt.float32
AF = mybir.ActivationFunctionType
DR = mybir.MatmulPerfMode.DoubleRow

GPERM = (3, 0, 1, 2)  # gate consumption order (tanh gate first)

# Stashed by kernel() so a test harness can read exec_time_ns / trace paths.
LAST_RESULTS = None


def _build_nc():
    nc = bacc.Bacc("TRN2", target_bir_lowering=False, debug=False,
                   num_devices=NCORES)

    combx_d = nc.dram_tensor("combx", [2, 128, KX * BS], F16,
                             kind="ExternalInput").ap()
    combh_d = nc.dram_tensor("combh", [2, 128, KH, BS], F8,
                             kind="ExternalInput").ap()
    combx8_d = nc.dram_tensor("combx8", [2, 128, KX, BS], F8,
                              kind="ExternalInput").ap()
    # g dimension pre-permuted host-side into consumption order (3,0,1,2).
    wx_d = nc.dram_tensor("wx", [2, HC, 4, 128, KX * 128], F16,
                          kind="ExternalInput").ap()
    wx8_d = nc.dram_tensor("wx8", [2, HC, 128, KX, 128], F8,
                           kind="ExternalInput").ap()
    wh_d = nc.dram_tensor("wh", [2, HC, 128, 4, KH, 128], F8,
                          kind="ExternalInput").ap()
    ct_d = nc.dram_tensor("ct", [2, HC, 128, BS], F32,
                          kind="ExternalInput").ap()
    bias_d = nc.dram_tensor("bias", [2, 128, 4 * HC], F32,
                            kind="ExternalInput").ap()
    hT_d = nc.dram_tensor("hT", [2, HC, 128, BS], F32,
                          kind="ExternalOutput").ap()
    cT_d = nc.dram_tensor("cT", [2, HC, 128, BS], F32,
                          kind="ExternalOutput").ap()

    with tile.TileContext(nc) as tc:
        with (
            tc.tile_pool(name="comb", bufs=2) as comb_pool,
            tc.tile_pool(name="w", bufs=6) as w_pool,
            tc.tile_pool(name="psum", bufs=6, space="PSUM") as psum_pool,
            tc.tile_pool(name="dpsum", bufs=2, space="PSUM") as dummy_pool,
            tc.tile_pool(name="gates", bufs=8) as gate_pool,
            tc.tile_pool(name="cc", bufs=3) as c_pool,
            tc.tile_pool(name="tmp", bufs=3) as tmp_pool,
            tc.tile_pool(name="biasp", bufs=2) as bias_pool,
        ):
            warm = comb_pool.tile([128, BS], F16, name="warm", tag="warm")
            nc.gpsimd.memset(warm[:], 0.0)

            def dummy_mms(n):
                for _ in range(n):
                    wps = dummy_pool.tile([128, BS], F32, name="wps",
                                          tag="wps")
                    nc.tensor.matmul(wps[:], warm[:, :128], warm[:],
                                     start=True, stop=True)

            # HAM warmup during the initial DMA fill; sized so the dummy
            # stream ends right as the first real matmul's data lands
            # (~8 run at the cold 1.2 GHz clock, the rest warm).
            dummy_mms(14)

            for d in range(2):
                combh = comb_pool.tile([128, KH, BS], F8, name="combh",
                                       tag="combh")
                if d == 0:
                    # three slices matching the j-major startup schedule:
                    # each slice unlocks one j-level (4 matmuls) of the
                    # h-part as it lands.
                    nc.scalar.dma_start(combh[:, :2, :],
                                        combh_d[d, :, :2, :])
                    nc.scalar.dma_start(combh[:, 2:4, :],
                                        combh_d[d, :, 2:4, :])
                    nc.scalar.dma_start(combh[:, 4:, :],
                                        combh_d[d, :, 4:, :])
                else:
                    nc.scalar.dma_start(combh[:, :KH // 2, :],
                                        combh_d[d, :, :KH // 2, :])
                    nc.scalar.dma_start(combh[:, KH // 2:, :],
                                        combh_d[d, :, KH // 2:, :])
                combx8 = comb_pool.tile([128, KX, BS], F8, name="combx8",
                                        tag="combx8")
                if d == 0:
                    nc.scalar.dma_start(combx8[:, :2, :],
                                        combx8_d[d, :, :2, :])
                    nc.scalar.dma_start(combx8[:, 2:, :],
                                        combx8_d[d, :, 2:, :])
                else:
                    nc.gpsimd.dma_start(combx8[:], combx8_d[d])
                bias_t = bias_pool.tile([128, 4 * HC], F32, name="bias_t",
                                        tag="bias_t")
                nc.gpsimd.dma_start(bias_t[:], bias_d[d])
                combxs = []
                for cc in range(2):
                    cb = comb_pool.tile([128, 4 * BS], F16,
                                        name=f"combx{cc}", tag=f"combx{cc}")
                    if d == 0 and cc == 0:
                        # k0 (128 KB) on the gpsimd ring in parallel with
                        # combh/combx8 on scalar; k1-3 behind them.
                        nc.gpsimd.dma_start(cb[:, :BS],
                                            combx_d[d, :, :BS])
                        nc.scalar.dma_start(cb[:, BS:],
                                            combx_d[d, :, BS:4 * BS])
                    elif d == 0:
                        nc.gpsimd.dma_start(cb[:],
                                            combx_d[d, :, 4 * BS:8 * BS])
                    else:
                        nc.scalar.dma_start(cb[:],
                                            combx_d[d, :, ts(cc, 4 * BS)])
                    combxs.append(cb)
                for hc in range(HC):
                    first = d == 0 and hc == 0
                    # all four gates' h-part (fp8 DR) weights in one
                    # 512 KB transfer with 4 KB lines (split for group 0
                    # so the first matmul gates on 128 KB).
                    wt8 = w_pool.tile([128, 4, KH, 128], F8,
                                      name="wt8", tag="wt8")
                    if first:
                        nc.sync.dma_start(wt8[:, 0], wh_d[d, hc, :, 0])
                        nc.sync.dma_start(wt8[:, 1:], wh_d[d, hc, :, 1:])
                    else:
                        nc.sync.dma_start(wt8[:], wh_d[d, hc])
                    wt8x = w_pool.tile([128, KX, 128], F8,
                                       name="wt8x", tag="wt8x")
                    nc.sync.dma_start(wt8x[:], wx8_d[d, hc])
                    ct = c_pool.tile([128, BS], F32, name="ct_t",
                                     tag="ct_t")
                    nc.gpsimd.dma_start(ct[:], ct_d[d, hc])
                    # fp16 x-weights alternate between the gpsimd and sync
                    # rings so the weight stream rides two rings in
                    # parallel; the last two groups pin to sync so the
                    # tail stores don't queue behind weight transfers on
                    # gpsimd.
                    if d == 1 and hc >= HC - 2:
                        w16_eng = nc.sync
                    else:
                        w16_eng = nc.gpsimd if hc % 2 == 0 else nc.sync
                    wts = {}
                    for gi, g in enumerate(GPERM):
                        if g == 0:
                            continue
                        wt = w_pool.tile([128, KX * 128], F16,
                                         name="wt", tag="wt")
                        w16_eng.dma_start(wt[:], wx_d[d, hc, gi])
                        wts[g] = wt
                    if first:
                        # --- startup group: h-parts of all four gates
                        # first (cheapest data), then f's fp8 x-part,
                        # then the fp16 x-parts; dummy matmuls keep the
                        # PE busy while the DMA rings ramp.
                        # j-major over gates: each combh slice feeds four
                        # matmuls (one per gate) the moment it lands, so
                        # the ramping scalar ring is never hit four times
                        # for the same missing slice.
                        pss = {}
                        for gi, g in enumerate(GPERM):
                            pss[g] = psum_pool.tile([128, BS], F32,
                                                    name="ps", tag="ps")
                        for j in range(KH // 2):
                            for gi, g in enumerate(GPERM):
                                nc.tensor.matmul(
                                    pss[g][:],
                                    wt8[:, gi, 2 * j:2 * j + 2, :],
                                    combh[:, 2 * j:2 * j + 2, :],
                                    start=(j == 0), stop=False,
                                    perf_mode=DR,
                                )
                            if j == 0:
                                dummy_mms(2)
                            elif j == 1:
                                dummy_mms(4)
                        for j in range(KX // 2):
                            nc.tensor.matmul(
                                pss[0][:], wt8x[:, 2 * j:2 * j + 2, :],
                                combx8[:, 2 * j:2 * j + 2, :],
                                start=False, stop=(j == KX // 2 - 1),
                                perf_mode=DR,
                            )
                        dummy_mms(2)
                        for g in (3, 1, 2):
                            for k in range(KX):
                                nc.tensor.matmul(
                                    pss[g][:], wts[g][:, ts(k, 128)],
                                    combxs[k // 4][:, ts(k % 4, BS)],
                                    start=False, stop=(k == KX - 1),
                                )
                            if g == 3:
                                dummy_mms(2)
                        gts = {}
                        for g in (0, 3, 1, 2):  # completion order
                            gt = gate_pool.tile([128, BS], F32, name="gt",
                                                tag="gt")
                            nc.scalar.activation(
                                gt[:], pss[g][:],
                                AF.Sigmoid if g < 3 else AF.Tanh,
                                bias=bias_t[:, g * HC + hc:
                                            g * HC + hc + 1],
                            )
                            gts[g] = gt
                    else:
                        gts = {}
                        # tanh gate (C) first so the post-matmul tail
                        # chain of the final group is short.
                        for gi, g in enumerate(GPERM):
                            if d == 1 and hc == HC - 1 and gi == 3:
                                # Final group: two half-N chains so the
                                # first half's ACT/DVE/store pipeline
                                # under the second half's matmuls.
                                halves = []
                                HB = BS // 2
                                for h2 in range(2):
                                    psH = psum_pool.tile(
                                        [128, HB], F32, name="psH",
                                        tag="ps")
                                    for j in range(KH // 2):
                                        nc.tensor.matmul(
                                            psH[:],
                                            wt8[:, gi, 2 * j:2 * j + 2, :],
                                            combh[:, 2 * j:2 * j + 2,
                                                  h2 * HB:(h2 + 1) * HB],
                                            start=(j == 0), stop=False,
                                            perf_mode=DR,
                                        )
                                    for k in range(KX):
                                        base = (k % 4) * BS + h2 * HB
                                        nc.tensor.matmul(
                                            psH[:],
                                            wts[g][:, ts(k, 128)],
                                            combxs[k // 4][:,
                                                           base:base + HB],
                                            start=False,
                                            stop=(k == KX - 1),
                                        )
                                    gtH = gate_pool.tile(
                                        [128, HB], F32, name="gtH",
                                        tag="gt")
                                    nc.scalar.activation(
                                        gtH[:], psH[:], AF.Sigmoid,
                                        bias=bias_t[:, g * HC + hc:
                                                    g * HC + hc + 1],
                                    )
                                    halves.append(gtH)
                                gts[g] = halves
                                continue
                            ps = psum_pool.tile([128, BS], F32, name="ps",
                                                tag="ps")
                            for j in range(KH // 2):
                                nc.tensor.matmul(
                                    ps[:], wt8[:, gi, 2 * j:2 * j + 2, :],
                                    combh[:, 2 * j:2 * j + 2, :],
                                    start=(j == 0), stop=False,
                                    perf_mode=DR,
                                )
                            if g == 0:
                                for j in range(KX // 2):
                                    nc.tensor.matmul(
                                        ps[:], wt8x[:, 2 * j:2 * j + 2, :],
                                        combx8[:, 2 * j:2 * j + 2, :],
                                        start=False,
                                        stop=(j == KX // 2 - 1),
                                        perf_mode=DR,
                                    )
                            else:
                                for k in range(KX):
                                    nc.tensor.matmul(
                                        ps[:], wts[g][:, ts(k, 128)],
                                        combxs[k // 4][:, ts(k % 4, BS)],
                                        start=False, stop=(k == KX - 1),
                                    )
                            gt = gate_pool.tile([128, BS], F32, name="gt",
                                                tag="gt")
                            nc.scalar.activation(
                                gt[:], ps[:],
                                AF.Sigmoid if g < 3 else AF.Tanh,
                                bias=bias_t[:, g * HC + hc:
                                            g * HC + hc + 1],
                            )
                            gts[g] = gt
                            # groups 1-2 still race the ramping weight
                            # stream; a few dummies bridge the gaps
                            # without letting the HAM re-throttle.
                            if d == 0 and hc == 1 and gi < 3:
                                dummy_mms(2)
                            elif d == 0 and hc == 2 and gi < 2:
                                dummy_mms(2)
                    gts = [gts[0], gts[1], gts[2], gts[3]]
                    last = d == 1 and hc == HC - 1
                    t1 = tmp_pool.tile([128, BS], F32, name="t1", tag="t1")
                    nc.vector.tensor_mul(t1[:], gts[0][:], ct[:])
                    t2 = tmp_pool.tile([128, BS], F32, name="t2", tag="t2")
                    nc.vector.tensor_mul(t2[:], gts[1][:], gts[3][:])
                    cnew = tmp_pool.tile([128, BS], F32, name="cnew",
                                         tag="cnew")
                    nc.vector.tensor_add(cnew[:], t1[:], t2[:])
                    tanhc = tmp_pool.tile([128, BS], F32, name="tanhc",
                                          tag="tanhc")
                    nc.scalar.activation(tanhc[:], cnew[:], AF.Tanh)
                    if last:
                        # tail: halves on two rings so the final stores
                        # drain in parallel instead of serializing.
                        HB = BS // 2
                        nc.sync.dma_start(cT_d[d, hc, :, :HB],
                                          cnew[:, :HB])
                        nc.gpsimd.dma_start(cT_d[d, hc, :, HB:],
                                            cnew[:, HB:])
                    else:
                        nc.scalar.dma_start(cT_d[d, hc], cnew[:])
                    if isinstance(gts[2], list):
                        HB = BS // 2
                        store_eng = (nc.scalar, nc.sync)
                        for h2, oH in enumerate(gts[2]):
                            hnH = tmp_pool.tile([128, HB], F32,
                                                name="hnH", tag="hnew")
                            nc.vector.tensor_mul(
                                hnH[:], oH[:],
                                tanhc[:, h2 * HB:(h2 + 1) * HB])
                            store_eng[h2].dma_start(
                                hT_d[d, hc, :, h2 * HB:(h2 + 1) * HB],
                                hnH[:])
                    else:
                        hnew = tmp_pool.tile([128, BS], F32, name="hnew",
                                             tag="hnew")
                        nc.vector.tensor_mul(hnew[:], gts[2][:], tanhc[:])
                        nc.scalar.dma_start(hT_d[d, hc], hnew[:])
    nc.compile()
    return nc


def _prep_w(W):
    # W [4, 1024, 2048] f32 (gate, h, i) -> (wx fp16, wx8 fp8, wh fp8):
    # wx  [HC, 4(perm), 128 i_local, KX*128 (k, h_local)] from i in [0, 1024)
    # wx8 [HC, 128 i_local, KX, 128 h_local]  f-gate slice of the same range
    # wh  [HC, 128 i_local, 4(perm), KH, 128 h_local]  from i in [1024, 2048)
    # so the lhsT tile for (gate, hc, k) has i on partitions, with the gate
    # dim pre-permuted to the kernel's consumption order.
    w5 = W.reshape(4, HC, 128, 16, 128).transpose(0, 1, 4, 3, 2)[list(GPERM)]
    # w5: [g(perm), hc, i_local, k(0..15), h_local]
    wx = np.ascontiguousarray(
        w5[:, :, :, :KX, :].transpose(1, 0, 2, 3, 4)
    ).astype(np.float16).reshape(HC, 4, 128, KX * 128)
    # f gate sits at permuted slot 1 (GPERM.index of gate 0)
    wx8 = np.ascontiguousarray(
        w5[1, :, :, :KX, :]
    ).astype(ml_dtypes.float8_e5m2)
    wh = np.ascontiguousarray(
        w5[:, :, :, KX:, :].transpose(1, 2, 0, 3, 4)
    ).astype(ml_dtypes.float8_e5m2)
    return wx, wx8, wh


def _prep_combx(x_slice):
    # [BS, 1024] f16 -> [128 i_local, KX*BS (k, b)]
    return np.ascontiguousarray(
        x_slice.T.reshape(KX, 128, BS).transpose(1, 0, 2)
    ).reshape(128, KX * BS)


def _prep_comb8(x_slice):
    # [BS, 1024] f32 -> fp8 [128 i_local, K, BS]
    return np.ascontiguousarray(
        x_slice.T.reshape(KX, 128, BS).transpose(1, 0, 2)
    ).astype(ml_dtypes.float8_e5m2)


def _prep_ct(c_slice):
    # [BS, 1024] f32 -> [HC, 128 h_local, BS]
    return np.ascontiguousarray(c_slice.T).reshape(HC, 128, BS)


def _prep_bias(b):
    # [4, 1024] f32 -> [128 h_local, 4*HC (g, hc)]
    return np.ascontiguousarray(
        b.reshape(4, HC, 128).transpose(2, 0, 1)
    ).reshape(128, 4 * HC)


def kernel(input_f, input_b, Hidden_State_f, Cell_State_f,
           Hidden_State_b, Cell_State_b, Wf, bf, Wb, bb):
    global LAST_RESULTS

    args = [np.asarray(a, dtype=np.float32) for a in (
        input_f, input_b, Hidden_State_f, Cell_State_f,
        Hidden_State_b, Cell_State_b, Wf, bf, Wb, bb)]
    (input_f, input_b, Hidden_State_f, Cell_State_f,
     Hidden_State_b, Cell_State_b, Wf, bf, Wb, bb) = args

    xf16 = input_f.astype(np.float16)
    xb16 = input_b.astype(np.float16)
    wxf, wx8f, whf = _prep_w(Wf)
    wxb, wx8b, whb = _prep_w(Wb)
    wx_all = np.stack([wxf, wxb])
    wx8_all = np.stack([wx8f, wx8b])
    wh_all = np.stack([whf, whb])
    bias_all = np.stack([_prep_bias(bf), _prep_bias(bb)])

    in_maps = []
    for c in range(NCORES):
        sl = slice(c * BS, (c + 1) * BS)
        in_maps.append({
            "combx": np.stack([_prep_combx(xf16[sl]), _prep_combx(xb16[sl])]),
            "combx8": np.stack([_prep_comb8(input_f[sl]),
                                _prep_comb8(input_b[sl])]),
            "combh": np.stack([_prep_comb8(Hidden_State_f[sl]),
                               _prep_comb8(Hidden_State_b[sl])]),
            "wx": wx_all,
            "wx8": wx8_all,
            "wh": wh_all,
            "ct": np.stack([_prep_ct(Cell_State_f[sl]),
                            _prep_ct(Cell_State_b[sl])]),
            "bias": bias_all,
        })

    nc = _build_nc()
    res = bass_utils.run_bass_kernel_spmd(nc, in_maps,
                                          core_ids=list(range(NCORES)))
    LAST_RESULTS = res

    h_f = np.empty((BATCH, HID), np.float32)
    c_f = np.empty((BATCH, HID), np.float32)
    h_b = np.empty((BATCH, HID), np.float32)
    c_b = np.empty((BATCH, HID), np.float32)
    for c in range(NCORES):
        sl = slice(c * BS, (c + 1) * BS)
        r = res.results[c]
        hT, cT = r["hT"], r["cT"]  # [2, HC, 128, BS] f32
        h_f[sl] = hT[0].reshape(HID, BS).T
        c_f[sl] = cT[0].reshape(HID, BS).T
        h_b[sl] = hT[1].reshape(HID, BS).T
        c_b[sl] = cT[1].reshape(HID, BS).T
    return h_f, c_f, h_b, c_b


# revision 26
# speedup vs baseline: 1.0291x; 1.0291x over previous
"""BiLSTM single-step kernel for 8 Trainium2 NeuronCores.

Math per direction d (f, b):
    gates    = x_d @ Wx_d^T + h_d @ Wh_d^T + b_d          # [4096, 4*1024]
    f,i,o    = sigmoid(...), C = tanh(...)
    c_new    = f*c + i*C ; h_new = o*tanh(c_new)

Distribution: data-parallel over batch, 512 rows per core; weights
replicated. Per core each direction is a [512, 2048] x [2048, 4096] GEMM.

Precision strategy: the x-part (|x|~1) runs in fp16 for the C/i/o gates;
the h-part is tiny (|h|~0.02, |h.Wh| ~ 2% of the gate magnitude) and runs
in fp8-e5m2 with DoubleRow perf mode (2 k-chunks per matmul instruction),
accumulating into the same fp32 PSUM bank. The f gate's x-part ALSO runs
in fp8-e5m2 DoubleRow: its gate output only multiplies the tiny cell
state (|c| <= ~0.1), so its error sensitivity is ~10x lower than the
other gates (CPU-verified end-to-end relmax ~5e-3, vs the 2e-2 budget).

On-chip layout is the transpose of the reference: psum tiles are
gates^T [128 gate-hidden partitions, 512 batch], so the per-(gate,h) bias
is per-partition (fused into the scalar-engine sigmoid/tanh) and the
contraction index i sits on SBUF partitions for both matmul operands.
All transposes happen host-side in numpy.

Schedule details (from perfetto analysis of earlier revisions):
  - The DMA rings take ~9 us to start flowing and are per-transfer
    latency-bound while ramping, so startup transfers are few and big
    (except a small first slice per ring to cut first-matmul latency).
  - Group 0 is inherently DMA-bound (~3.4 MB of weights+activations vs
    ~9.5 us of matmul time): it runs all four gates' fp8 h-parts first
    (cheapest data), then the f gate's fp8 x-part, then the fp16
    x-parts, with dummy matmuls on a zeroed tile interleaved so the
    PE_HAM clock gate never sees an idle window (a re-throttle to
    1.2 GHz costs ~3-5 us; the dummies run where the PE would stall).
  - ~10 dummy matmuls also run during the initial pure-DMA fill so the
    PE is already at 2.4 GHz when the first real matmul issues.
  - Per-group fp8 weights ride in one 512 KB transfer (4 KB contiguous
    lines) instead of four 128 KB ones (1 KB lines).
  - The final group's c/h stores are split across the three DMA rings
    instead of serializing ~512 KB on one ring at the kernel tail.
"""

import numpy as np
import ml_dtypes

import concourse.bass as bass
import concourse.mybir as mybir
import concourse.tile as tile
from concourse import bacc, bass_utils
from concourse.bass import ts

BATCH, IN, HID = 4096, 1024, 1024
NCORES = 8
BS = BATCH // NCORES          # 512 batch rows per core = matmul free dim N
KX = IN // 128                # 8 contraction chunks (x part)
KH = HID // 128               # 8 fp8 contraction chunks (h part)
HC = HID // 128               # 8 hidden chunks of 128

F16 = mybir.dt.float16
F8 = mybir.dt.float8e5
F32 = mybir.dt.float32
AF = mybir.ActivationFunctionType
DR = mybir.MatmulPerfMode.DoubleRow

GPERM = (3, 0, 1, 2)  # gate consumption order (tanh gate first)

# Stashed by kernel() so a test harness can read exec_time_ns / trace paths.
LAST_RESULTS = None


def _build_nc():
    nc = bacc.Bacc("TRN2", target_bir_lowering=False, debug=False,
                   num_devices=NCORES)

    combx_d = nc.dram_tensor("combx", [2, 128, KX * BS], F16,
                             kind="ExternalInput").ap()
    combh_d = nc.dram_tensor("combh", [2, 128, KH, BS], F8,
                             kind="ExternalInput").ap()
    combx8_d = nc.dram_tensor("combx8", [2, 128, KX, BS], F8,
                              kind="ExternalInput").ap()
    # g dimension pre-permuted host-side into consumption order (3,0,1,2).
    wx_d = nc.dram_tensor("wx", [2, HC, 4, 128, KX * 128], F16,
                          kind="ExternalInput").ap()
    wx8_d = nc.dram_tensor("wx8", [2, HC, 128, KX, 128], F8,
                           kind="ExternalInput").ap()
    wh_d = nc.dram_tensor("wh", [2, HC, 128, 4, KH, 128], F8,
                          kind="ExternalInput").ap()
    ct_d = nc.dram_tensor("ct", [2, HC, 128, BS], F32,
                          kind="ExternalInput").ap()
    bias_d = nc.dram_tensor("bias", [2, 128, 4 * HC], F32,
                            kind="ExternalInput").ap()
    hT_d = nc.dram_tensor("hT", [2, HC, 128, BS], F32,
                          kind="ExternalOutput").ap()
    cT_d = nc.dram_tensor("cT", [2, HC, 128, BS], F32,
                          kind="ExternalOutput").ap()

    with tile.TileContext(nc) as tc:
        with (
            tc.tile_pool(name="comb", bufs=2) as comb_pool,
            tc.tile_pool(name="w", bufs=6) as w_pool,
            tc.tile_pool(name="psum", bufs=6, space="PSUM") as psum_pool,
            tc.tile_pool(name="dpsum", bufs=2, space="PSUM") as dummy_pool,
            tc.tile_pool(name="gates", bufs=8) as gate_pool,
            tc.tile_pool(name="cc", bufs=3) as c_pool,
            tc.tile_pool(name="tmp", bufs=3) as tmp_pool,
            tc.tile_pool(name="biasp", bufs=2) as bias_pool,
        ):
            warm = comb_pool.tile([128, BS], F16, name="warm", tag="warm")
            nc.gpsimd.memset(warm[:], 0.0)

            def dummy_mms(n):
                for _ in range(n):
                    wps = dummy_pool.tile([128, BS], F32, name="wps",
                                          tag="wps")
                    nc.tensor.matmul(wps[:], warm[:, :128], warm[:],
                                     start=True, stop=True)

            # HAM warmup during the initial DMA fill; sized so the dummy
            # stream ends right as the first real matmul's data lands
            # (~8 run at the cold 1.2 GHz clock, the rest warm).
            dummy_mms(14)

            for d in range(2):
                combh = comb_pool.tile([128, KH, BS], F8, name="combh",
                                       tag="combh")
                if d == 0:
                    # three slices matching the j-major startup schedule:
                    # each slice unlocks one j-level (4 matmuls) of the
                    # h-part as it lands.
                    nc.scalar.dma_start(combh[:, :2, :],
                                        combh_d[d, :, :2, :])
                    nc.scalar.dma_start(combh[:, 2:4, :],
                                        combh_d[d, :, 2:4, :])
                    nc.scalar.dma_start(combh[:, 4:, :],
                                        combh_d[d, :, 4:, :])
                else:
                    nc.scalar.dma_start(combh[:, :KH // 2, :],
                                        combh_d[d, :, :KH // 2, :])
                    nc.scalar.dma_start(combh[:, KH // 2:, :],
                                        combh_d[d, :, KH // 2:, :])
                combx8 = comb_pool.tile([128, KX, BS], F8, name="combx8",
                                        tag="combx8")
                if d == 0:
                    nc.scalar.dma_start(combx8[:, :4, :],
                                        combx8_d[d, :, :4, :])
                    nc.scalar.dma_start(combx8[:, 4:, :],
                                        combx8_d[d, :, 4:, :])
                else:
                    nc.gpsimd.dma_start(combx8[:], combx8_d[d])
                bias_t = bias_pool.tile([128, 4 * HC], F32, name="bias_t",
                                        tag="bias_t")
                nc.gpsimd.dma_start(bias_t[:], bias_d[d])
                combxs = []
                cc1_pending = None
                for cc in range(2):
                    cb = comb_pool.tile([128, 4 * BS], F16,
                                        name=f"combx{cc}", tag=f"combx{cc}")
                    if d == 0 and cc == 0:
                        # k0 (128 KB) on the gpsimd ring in parallel with
                        # combh/combx8 on scalar; k1-3 behind them.
                        nc.gpsimd.dma_start(cb[:, :BS],
                                            combx_d[d, :, :BS])
                        nc.scalar.dma_start(cb[:, BS:],
                                            combx_d[d, :, BS:4 * BS])
                    elif d == 0:
                        # deferred: issued on gpsimd after group 0's fp16
                        # weights (which the k-major x-phase needs first).
                        cc1_pending = cb
                    else:
                        nc.scalar.dma_start(cb[:],
                                            combx_d[d, :, ts(cc, 4 * BS)])
                    combxs.append(cb)
                for hc in range(HC):
                    first = d == 0 and hc == 0
                    # all four gates' h-part (fp8 DR) weights in one
                    # 512 KB transfer with 4 KB lines (split for group 0
                    # so the first matmul gates on 128 KB).
                    wt8 = w_pool.tile([128, 4, KH, 128], F8,
                                      name="wt8", tag="wt8")
                    if first:
                        nc.sync.dma_start(wt8[:, 0], wh_d[d, hc, :, 0])
                        nc.sync.dma_start(wt8[:, 1:], wh_d[d, hc, :, 1:])
                    else:
                        nc.sync.dma_start(wt8[:], wh_d[d, hc])
                    wt8x = w_pool.tile([128, KX, 128], F8,
                                       name="wt8x", tag="wt8x")
                    nc.sync.dma_start(wt8x[:], wx8_d[d, hc])
                    ct = c_pool.tile([128, BS], F32, name="ct_t",
                                     tag="ct_t")
                    if not first:
                        nc.gpsimd.dma_start(ct[:], ct_d[d, hc])
                    # fp16 x-weights alternate between the gpsimd and sync
                    # rings so the weight stream rides two rings in
                    # parallel; the last two groups pin to sync so the
                    # tail stores don't queue behind weight transfers on
                    # gpsimd.
                    if d == 1 and hc >= HC - 2:
                        w16_eng = nc.sync
                    else:
                        w16_eng = nc.gpsimd if hc % 2 == 0 else nc.sync
                    wts = {}
                    for gi, g in enumerate(GPERM):
                        if g == 0:
                            continue
                        wt = w_pool.tile([128, KX * 128], F16,
                                         name="wt", tag="wt")
                        w16_eng.dma_start(wt[:], wx_d[d, hc, gi])
                        wts[g] = wt
                    if first:
                        # gpsimd queue order for startup: k0, fp16
                        # weights (k-major x-phase needs all three
                        # early), then the second combx half, then ct.
                        nc.gpsimd.dma_start(combxs[1][:],
                                            combx_d[d, :, 4 * BS:8 * BS])
                        cc1_pending = None
                        nc.gpsimd.dma_start(ct[:], ct_d[d, hc])
                    if first:
                        # --- startup group: h-parts of all four gates
                        # first (cheapest data), then f's fp8 x-part,
                        # then the fp16 x-parts; dummy matmuls keep the
                        # PE busy while the DMA rings ramp.
                        # j-major over gates: each combh slice feeds four
                        # matmuls (one per gate) the moment it lands, so
                        # the ramping scalar ring is never hit four times
                        # for the same missing slice.
                        pss = {}
                        for gi, g in enumerate(GPERM):
                            pss[g] = psum_pool.tile([128, BS], F32,
                                                    name="ps", tag="ps")
                        for j in range(KH // 2):
                            for gi, g in enumerate(GPERM):
                                nc.tensor.matmul(
                                    pss[g][:],
                                    wt8[:, gi, 2 * j:2 * j + 2, :],
                                    combh[:, 2 * j:2 * j + 2, :],
                                    start=(j == 0), stop=False,
                                    perf_mode=DR,
                                )
                            if j == 0:
                                dummy_mms(2)
                            elif j == 1:
                                dummy_mms(4)
                        for j in range(KX // 2):
                            nc.tensor.matmul(
                                pss[0][:], wt8x[:, 2 * j:2 * j + 2, :],
                                combx8[:, 2 * j:2 * j + 2, :],
                                start=False, stop=(j == KX // 2 - 1),
                                perf_mode=DR,
                            )
                            if j == 1:
                                dummy_mms(2)
                        dummy_mms(2)
                        # k-major x-phase: combx chunks are consumed in
                        # strict arrival order, three matmuls per chunk.
                        for k in range(KX):
                            for g in (3, 1, 2):
                                nc.tensor.matmul(
                                    pss[g][:], wts[g][:, ts(k, 128)],
                                    combxs[k // 4][:, ts(k % 4, BS)],
                                    start=False, stop=(k == KX - 1),
                                )
                        gts = {}
                        for g in (0, 3, 1, 2):  # completion order
                            gt = gate_pool.tile([128, BS], F32, name="gt",
                                                tag="gt")
                            nc.scalar.activation(
                                gt[:], pss[g][:],
                                AF.Sigmoid if g < 3 else AF.Tanh,
                                bias=bias_t[:, g * HC + hc:
                                            g * HC + hc + 1],
                            )
                            gts[g] = gt
                    else:
                        gts = {}
                        # tanh gate (C) first so the post-matmul tail
                        # chain of the final group is short.
                        for gi, g in enumerate(GPERM):
                            if d == 1 and hc == HC - 1 and gi == 3:
                                # Final group: two half-N chains so the
                                # first half's ACT/DVE/store pipeline
                                # under the second half's matmuls.
                                halves = []
                                HB = BS // 2
                                for h2 in range(2):
                                    psH = psum_pool.tile(
                                        [128, HB], F32, name="psH",
                                        tag="ps")
                                    for j in range(KH // 2):
                                        nc.tensor.matmul(
                                            psH[:],
                                            wt8[:, gi, 2 * j:2 * j + 2, :],
                                            combh[:, 2 * j:2 * j + 2,
                                                  h2 * HB:(h2 + 1) * HB],
                                            start=(j == 0), stop=False,
                                            perf_mode=DR,
                                        )
                                    for k in range(KX):
                                        base = (k % 4) * BS + h2 * HB
                                        nc.tensor.matmul(
                                            psH[:],
                                            wts[g][:, ts(k, 128)],
                                            combxs[k // 4][:,
                                                           base:base + HB],
                                            start=False,
                                            stop=(k == KX - 1),
                                        )
                                    gtH = gate_pool.tile(
                                        [128, HB], F32, name="gtH",
                                        tag="gt")
                                    nc.scalar.activation(
                                        gtH[:], psH[:], AF.Sigmoid,
                                        bias=bias_t[:, g * HC + hc:
                                                    g * HC + hc + 1],
                                    )
                                    halves.append(gtH)
                                gts[g] = halves
                                continue
                            ps = psum_pool.tile([128, BS], F32, name="ps",
                                                tag="ps")
                            for j in range(KH // 2):
                                nc.tensor.matmul(
                                    ps[:], wt8[:, gi, 2 * j:2 * j + 2, :],
                                    combh[:, 2 * j:2 * j + 2, :],
                                    start=(j == 0), stop=False,
                                    perf_mode=DR,
                                )
                            if g == 0:
                                for j in range(KX // 2):
                                    nc.tensor.matmul(
                                        ps[:], wt8x[:, 2 * j:2 * j + 2, :],
                                        combx8[:, 2 * j:2 * j + 2, :],
                                        start=False,
                                        stop=(j == KX // 2 - 1),
                                        perf_mode=DR,
                                    )
                            else:
                                for k in range(KX):
                                    nc.tensor.matmul(
                                        ps[:], wts[g][:, ts(k, 128)],
                                        combxs[k // 4][:, ts(k % 4, BS)],
                                        start=False, stop=(k == KX - 1),
                                    )
                            gt = gate_pool.tile([128, BS], F32, name="gt",
                                                tag="gt")
                            nc.scalar.activation(
                                gt[:], ps[:],
                                AF.Sigmoid if g < 3 else AF.Tanh,
                                bias=bias_t[:, g * HC + hc:
                                            g * HC + hc + 1],
                            )
                            gts[g] = gt
                            # groups 1-2 still race the ramping weight
                            # stream; a few dummies bridge the gaps
                            # without letting the HAM re-throttle.
                            if d == 0 and hc in (1, 2) and gi == 0:
                                dummy_mms(2)
                    gts = [gts[0], gts[1], gts[2], gts[3]]
                    last = d == 1 and hc == HC - 1
                    t1 = tmp_pool.tile([128, BS], F32, name="t1", tag="t1")
                    nc.vector.tensor_mul(t1[:], gts[0][:], ct[:])
                    t2 = tmp_pool.tile([128, BS], F32, name="t2", tag="t2")
                    nc.vector.tensor_mul(t2[:], gts[1][:], gts[3][:])
                    cnew = tmp_pool.tile([128, BS], F32, name="cnew",
                                         tag="cnew")
                    nc.vector.tensor_add(cnew[:], t1[:], t2[:])
                    tanhc = tmp_pool.tile([128, BS], F32, name="tanhc",
                                          tag="tanhc")
                    nc.scalar.activation(tanhc[:], cnew[:], AF.Tanh)
                    if last:
                        # tail: halves on two rings so the final stores
                        # drain in parallel instead of serializing.
                        HB = BS // 2
                        nc.sync.dma_start(cT_d[d, hc, :, :HB],
                                          cnew[:, :HB])
                        nc.gpsimd.dma_start(cT_d[d, hc, :, HB:],
                                            cnew[:, HB:])
                    else:
                        nc.scalar.dma_start(cT_d[d, hc], cnew[:])
                    if isinstance(gts[2], list):
                        HB = BS // 2
                        store_eng = (nc.scalar, nc.sync)
                        for h2, oH in enumerate(gts[2]):
                            hnH = tmp_pool.tile([128, HB], F32,
                                                name="hnH", tag="hnew")
                            nc.vector.tensor_mul(
                                hnH[:], oH[:],
                                tanhc[:, h2 * HB:(h2 + 1) * HB])
                            store_eng[h2].dma_start(
                                hT_d[d, hc, :, h2 * HB:(h2 + 1) * HB],
                                hnH[:])
                    else:
                        hnew = tmp_pool.tile([128, BS], F32, name="hnew",
                                             tag="hnew")
                        nc.vector.tensor_mul(hnew[:], gts[2][:], tanhc[:])
                        nc.scalar.dma_start(hT_d[d, hc], hnew[:])
    nc.compile()
    return nc


def _prep_w(W):
    # W [4, 1024, 2048] f32 (gate, h, i) -> (wx fp16, wx8 fp8, wh fp8):
    # wx  [HC, 4(perm), 128 i_local, KX*128 (k, h_local)] from i in [0, 1024)
    # wx8 [HC, 128 i_local, KX, 128 h_local]  f-gate slice of the same range
    # wh  [HC, 128 i_local, 4(perm), KH, 128 h_local]  from i in [1024, 2048)
    # so the lhsT tile for (gate, hc, k) has i on partitions, with the gate
    # dim pre-permuted to the kernel's consumption order.
    w5 = W.reshape(4, HC, 128, 16, 128).transpose(0, 1, 4, 3, 2)[list(GPERM)]
    # w5: [g(perm), hc, i_local, k(0..15), h_local]
    wx = np.ascontiguousarray(
        w5[:, :, :, :KX, :].transpose(1, 0, 2, 3, 4)
    ).astype(np.float16).reshape(HC, 4, 128, KX * 128)
    # f gate sits at permuted slot 1 (GPERM.index of gate 0)
    wx8 = np.ascontiguousarray(
        w5[1, :, :, :KX, :]
    ).astype(ml_dtypes.float8_e5m2)
    wh = np.ascontiguousarray(
        w5[:, :, :, KX:, :].transpose(1, 2, 0, 3, 4)
    ).astype(ml_dtypes.float8_e5m2)
    return wx, wx8, wh


def _prep_combx(x_slice):
    # [BS, 1024] f16 -> [128 i_local, KX*BS (k, b)]
    return np.ascontiguousarray(
        x_slice.T.reshape(KX, 128, BS).transpose(1, 0, 2)
    ).reshape(128, KX * BS)


def _prep_comb8(x_slice):
    # [BS, 1024] f32 -> fp8 [128 i_local, K, BS]
    return np.ascontiguousarray(
        x_slice.T.reshape(KX, 128, BS).transpose(1, 0, 2)
    ).astype(ml_dtypes.float8_e5m2)


def _prep_ct(c_slice):
    # [BS, 1024] f32 -> [HC, 128 h_local, BS]
    return np.ascontiguousarray(c_slice.T).reshape(HC, 128, BS)


def _prep_bias(b):
    # [4, 1024] f32 -> [128 h_local, 4*HC (g, hc)]
    return np.ascontiguousarray(
        b.reshape(4, HC, 128).transpose(2, 0, 1)
    ).reshape(128, 4 * HC)


def kernel(input_f, input_b, Hidden_State_f, Cell_State_f,
           Hidden_State_b, Cell_State_b, Wf, bf, Wb, bb):
    global LAST_RESULTS

    args = [np.asarray(a, dtype=np.float32) for a in (
        input_f, input_b, Hidden_State_f, Cell_State_f,
        Hidden_State_b, Cell_State_b, Wf, bf, Wb, bb)]
    (input_f, input_b, Hidden_State_f, Cell_State_f,
     Hidden_State_b, Cell_State_b, Wf, bf, Wb, bb) = args

    xf16 = input_f.astype(np.float16)
    xb16 = input_b.astype(np.float16)
    wxf, wx8f, whf = _prep_w(Wf)
    wxb, wx8b, whb = _prep_w(Wb)
    wx_all = np.stack([wxf, wxb])
    wx8_all = np.stack([wx8f, wx8b])
    wh_all = np.stack([whf, whb])
    bias_all = np.stack([_prep_bias(bf), _prep_bias(bb)])

    in_maps = []
    for c in range(NCORES):
        sl = slice(c * BS, (c + 1) * BS)
        in_maps.append({
            "combx": np.stack([_prep_combx(xf16[sl]), _prep_combx(xb16[sl])]),
            "combx8": np.stack([_prep_comb8(input_f[sl]),
                                _prep_comb8(input_b[sl])]),
            "combh": np.stack([_prep_comb8(Hidden_State_f[sl]),
                               _prep_comb8(Hidden_State_b[sl])]),
            "wx": wx_all,
            "wx8": wx8_all,
            "wh": wh_all,
            "ct": np.stack([_prep_ct(Cell_State_f[sl]),
                            _prep_ct(Cell_State_b[sl])]),
            "bias": bias_all,
        })

    nc = _build_nc()
    res = bass_utils.run_bass_kernel_spmd(nc, in_maps,
                                          core_ids=list(range(NCORES)))
    LAST_RESULTS = res

    h_f = np.empty((BATCH, HID), np.float32)
    c_f = np.empty((BATCH, HID), np.float32)
    h_b = np.empty((BATCH, HID), np.float32)
    c_b = np.empty((BATCH, HID), np.float32)
    for c in range(NCORES):
        sl = slice(c * BS, (c + 1) * BS)
        r = res.results[c]
        hT, cT = r["hT"], r["cT"]  # [2, HC, 128, BS] f32
        h_f[sl] = hT[0].reshape(HID, BS).T
        c_f[sl] = cT[0].reshape(HID, BS).T
        h_b[sl] = hT[1].reshape(HID, BS).T
        c_b[sl] = cT[1].reshape(HID, BS).T
    return h_f, c_f, h_b, c_b
